# revision 1
# baseline (speedup 1.0000x reference)
"""GroupQuantLinear: y = x @ dequant(w).T + b on 8 NeuronCores.

Strategy (column-parallel / tensor-parallel over out_features):
  - Host: dequantize packed 4-bit weights -> W (out,in) fp32, cast to bf16,
    pre-transpose to WT (in,out); pre-transpose x -> xT (in,tokens) bf16.
  - Shard WT / bias along out_features across 8 cores (1376 each).
  - Each core: WT shard resident in SBUF (bf16, 11.3MB); stream 128-token
    tiles of xT; bf16 matmuls accumulate over K=4096 in fp32 PSUM
    (4 PSUM banks x 344 out-cols per token tile); add bias on copy-out;
    DMA fp32 output in natural (tokens, outs) layout.
  - Host: concatenate the 8 output shards along the out dim.
"""

import os
import sys
from contextlib import ExitStack

import numpy as np

sys.path.insert(0, "/opt/trn_rl_repo")

TOKENS = 8192
IN_F = 4096
OUT_F = 11008
N_CORES = 8
SHARD = OUT_F // N_CORES          # 1376
NCH = 4
CH = SHARD // NCH                 # 344
P = 128
KS = IN_F // P                    # 32
TT = TOKENS // P                  # 64

_NC_CACHE = {}


def _build_nc():
    import concourse.bacc as bacc
    import concourse.mybir as mybir
    import concourse.tile as tile

    nc = bacc.Bacc(
        "TRN2",
        target_bir_lowering=False,
        debug=False,
        enable_asserts=False,
        num_devices=N_CORES,
    )
    xt = nc.dram_tensor("xt", (IN_F, TOKENS), mybir.dt.bfloat16, kind="ExternalInput").ap()
    wt = nc.dram_tensor("wt", (IN_F, SHARD), mybir.dt.bfloat16, kind="ExternalInput").ap()
    brep = nc.dram_tensor("brep", (P, SHARD), mybir.dt.float32, kind="ExternalInput").ap()
    y = nc.dram_tensor("y", (TOKENS, SHARD), mybir.dt.float32, kind="ExternalOutput").ap()

    with tile.TileContext(nc) as tc, ExitStack() as ctx:
        wpool = ctx.enter_context(tc.tile_pool(name="w", bufs=1))
        xpool = ctx.enter_context(tc.tile_pool(name="x", bufs=4))
        opool = ctx.enter_context(tc.tile_pool(name="o", bufs=8))
        pspool = ctx.enter_context(tc.tile_pool(name="ps", bufs=8, space="PSUM"))

        w_sb = wpool.tile([P, KS, SHARD], mybir.dt.bfloat16, name="w_sb")
        bias_sb = wpool.tile([P, SHARD], mybir.dt.float32, name="bias_sb")

        wt_r = wt.rearrange("(ks p) n -> p ks n", p=P)
        # Split the 11.3MB weight load across several DMA queues.
        for c in range(NCH):
            nc.sync.dma_start(
                w_sb[:, :, c * CH:(c + 1) * CH], wt_r[:, :, c * CH:(c + 1) * CH]
            )
        nc.sync.dma_start(bias_sb[:], brep)

        xt_r = xt.rearrange("(ks p) m -> p ks m", p=P)
        for t in range(TT):
            x_sb = xpool.tile([P, KS, P], mybir.dt.bfloat16, name="x_sb", tag="x_sb")
            nc.sync.dma_start(x_sb[:], xt_r[:, :, t * P:(t + 1) * P])

            pss = [
                pspool.tile([P, CH], mybir.dt.float32, name=f"ps{c}", tag="ps")
                for c in range(NCH)
            ]
            for ks in range(KS):
                for c in range(NCH):
                    nc.tensor.matmul(
                        pss[c][:],
                        x_sb[:, ks, :],
                        w_sb[:, ks, c * CH:(c + 1) * CH],
                        start=(ks == 0),
                        stop=(ks == KS - 1),
                    )
            for c in range(NCH):
                o_sb = opool.tile([P, CH], mybir.dt.float32, name="o_sb", tag="o_sb")
                nc.vector.tensor_add(o_sb[:], pss[c][:], bias_sb[:, c * CH:(c + 1) * CH])
                nc.sync.dma_start(y[t * P:(t + 1) * P, c * CH:(c + 1) * CH], o_sb[:])

    nc.compile()
    return nc


def _host_prep(x, w_packed, w_scale, w_bias, b):
    import ml_dtypes

    # Dequantize on host exactly as the reference does, then cast to bf16.
    shifts = np.array([12, 8, 4, 0], dtype=np.int32)
    nib = ((w_packed[..., None] >> shifts) & 15).astype(np.float32)
    n_rows, n_groups, n_ids = w_packed.shape
    W = nib.reshape(n_rows, n_groups, n_ids * 4) * w_scale + w_bias
    W = W.reshape(n_rows, n_groups * n_ids * 4)          # (out, in) fp32
    WT = np.ascontiguousarray(W.T.astype(ml_dtypes.bfloat16))   # (in, out) bf16
    xT = np.ascontiguousarray(x.T.astype(ml_dtypes.bfloat16))   # (in, tokens) bf16

    in_maps = []
    for i in range(N_CORES):
        sl = slice(i * SHARD, (i + 1) * SHARD)
        in_maps.append(
            {
                "xt": xT,
                "wt": np.ascontiguousarray(WT[:, sl]),
                "brep": np.ascontiguousarray(
                    np.broadcast_to(b[sl].astype(np.float32), (P, SHARD))
                ),
            }
        )
    return in_maps


def _run(x, w_packed, w_scale, w_bias, b, trace=False):
    from concourse.bass_utils import run_bass_kernel_spmd

    if "nc" not in _NC_CACHE:
        _NC_CACHE["nc"] = _build_nc()
    nc = _NC_CACHE["nc"]
    in_maps = _host_prep(x, w_packed, w_scale, w_bias, b)
    res = run_bass_kernel_spmd(nc, in_maps, list(range(N_CORES)), trace=trace)
    y = np.concatenate([res.results[i]["y"] for i in range(N_CORES)], axis=1)
    return np.ascontiguousarray(y.astype(np.float32)), res


def kernel(x, w_packed, w_scale, w_bias, b):
    y, _ = _run(x, w_packed, w_scale, w_bias, b, trace=False)
    return y


# revision 3
# speedup vs baseline: 1.0222x; 1.0222x over previous
"""GroupQuantLinear: y = x @ dequant(w).T + b on 8 NeuronCores.

Strategy (column-parallel / tensor-parallel over out_features):
  - Host: dequantize packed 4-bit weights -> W (out,in) fp32, cast to fp16,
    pre-transpose to WT (in,out); pre-transpose x -> xT (in,tokens) fp16.
  - Shard WT / bias along out_features across 8 cores (1376 each).
  - Each core: WT shard resident in SBUF (fp16, 11.3MB); stream 128-token
    tiles of xT; fp16 matmuls accumulate over K=4096 in fp32 PSUM
    (3 PSUM banks: 512/512/352 out-cols per token tile); add bias on
    copy-out; DMA fp32 output in natural (tokens, outs) layout.
  - W is loaded in ks-major slabs so the PE can start after ~2MB arrives.
  - Host: concatenate the 8 output shards along the out dim.
"""

import os
import sys
from contextlib import ExitStack

import numpy as np

sys.path.insert(0, "/opt/trn_rl_repo")

TOKENS = 8192
IN_F = 4096
OUT_F = 11008
N_CORES = 8
SHARD = OUT_F // N_CORES          # 1376
CHUNKS = (512, 512, 352)          # out-cols per PSUM bank, sum = SHARD
P = 128
KS = IN_F // P                    # 32
TT = TOKENS // P                  # 64
W_SLAB = 2                        # ks per W-load DMA slab

_NC_CACHE = {}


def _build_nc():
    import concourse.bacc as bacc
    import concourse.mybir as mybir
    import concourse.tile as tile

    dt16 = mybir.dt.float16

    nc = bacc.Bacc(
        "TRN2",
        target_bir_lowering=False,
        debug=False,
        enable_asserts=False,
        num_devices=N_CORES,
    )
    xt = nc.dram_tensor("xt", (IN_F, TOKENS), dt16, kind="ExternalInput").ap()
    wt = nc.dram_tensor("wt", (IN_F, SHARD), dt16, kind="ExternalInput").ap()
    brep = nc.dram_tensor("brep", (P, SHARD), mybir.dt.float32, kind="ExternalInput").ap()
    y = nc.dram_tensor("y", (TOKENS, SHARD), mybir.dt.float32, kind="ExternalOutput").ap()

    coff = [0]
    for ch in CHUNKS:
        coff.append(coff[-1] + ch)

    with tile.TileContext(nc) as tc, ExitStack() as ctx:
        wpool = ctx.enter_context(tc.tile_pool(name="w", bufs=1))
        xpool = ctx.enter_context(tc.tile_pool(name="x", bufs=4))
        opool = ctx.enter_context(tc.tile_pool(name="o", bufs=6))
        pspool = ctx.enter_context(tc.tile_pool(name="ps", bufs=2, space="PSUM"))

        w_sb = wpool.tile([P, KS, SHARD], dt16, name="w_sb")
        bias_sb = wpool.tile([P, SHARD], mybir.dt.float32, name="bias_sb")

        xt_r = xt.rearrange("(ks p) m -> p ks m", p=P)
        wt_r = wt.rearrange("(ks p) n -> p ks n", p=P)

        # First x tile before the weight slabs so the PE can start ASAP.
        x0 = xpool.tile([P, KS, P], dt16, name="x_sb", tag="x_sb")
        nc.sync.dma_start(x0[:], xt_r[:, :, 0:P])
        # W in ks-major slabs: PE consumes ks-sequentially during t=0.
        for s in range(0, KS, W_SLAB):
            nc.sync.dma_start(
                w_sb[:, s:s + W_SLAB, :], wt_r[:, s:s + W_SLAB, :]
            )
        nc.sync.dma_start(bias_sb[:], brep)

        for t in range(TT):
            if t == 0:
                x_sb = x0
            else:
                x_sb = xpool.tile([P, KS, P], dt16, name="x_sb", tag="x_sb")
                nc.sync.dma_start(x_sb[:], xt_r[:, :, t * P:(t + 1) * P])

            pss = [
                pspool.tile([P, CHUNKS[c]], mybir.dt.float32,
                            name=f"ps{c}", tag=f"ps{c}")
                for c in range(len(CHUNKS))
            ]
            for ks in range(KS):
                for c in range(len(CHUNKS)):
                    nc.tensor.matmul(
                        pss[c][:],
                        x_sb[:, ks, :],
                        w_sb[:, ks, coff[c]:coff[c + 1]],
                        start=(ks == 0),
                        stop=(ks == KS - 1),
                    )
            for c in range(len(CHUNKS)):
                o_sb = opool.tile([P, 512], mybir.dt.float32,
                                  name="o_sb", tag="o_sb")[:, :CHUNKS[c]]
                nc.vector.tensor_add(o_sb[:], pss[c][:], bias_sb[:, coff[c]:coff[c + 1]])
                nc.sync.dma_start(y[t * P:(t + 1) * P, coff[c]:coff[c + 1]], o_sb[:])

    nc.compile()
    return nc


def _host_prep(x, w_packed, w_scale, w_bias, b):
    import ml_dtypes  # noqa: F401

    # Dequantize on host exactly as the reference does, then cast to fp16.
    shifts = np.array([12, 8, 4, 0], dtype=np.int32)
    nib = ((w_packed[..., None] >> shifts) & 15).astype(np.float32)
    n_rows, n_groups, n_ids = w_packed.shape
    W = nib.reshape(n_rows, n_groups, n_ids * 4) * w_scale + w_bias
    W = W.reshape(n_rows, n_groups * n_ids * 4)          # (out, in) fp32
    WT = np.ascontiguousarray(W.T.astype(np.float16))    # (in, out) fp16
    xT = np.ascontiguousarray(x.T.astype(np.float16))    # (in, tokens) fp16

    in_maps = []
    for i in range(N_CORES):
        sl = slice(i * SHARD, (i + 1) * SHARD)
        in_maps.append(
            {
                "xt": xT,
                "wt": np.ascontiguousarray(WT[:, sl]),
                "brep": np.ascontiguousarray(
                    np.broadcast_to(b[sl].astype(np.float32), (P, SHARD))
                ),
            }
        )
    return in_maps


def _run(x, w_packed, w_scale, w_bias, b, trace=False):
    from concourse.bass_utils import run_bass_kernel_spmd

    if "nc" not in _NC_CACHE:
        _NC_CACHE["nc"] = _build_nc()
    nc = _NC_CACHE["nc"]
    in_maps = _host_prep(x, w_packed, w_scale, w_bias, b)
    res = run_bass_kernel_spmd(nc, in_maps, list(range(N_CORES)), trace=trace)
    y = np.concatenate([res.results[i]["y"] for i in range(N_CORES)], axis=1)
    return np.ascontiguousarray(y.astype(np.float32)), res


def kernel(x, w_packed, w_scale, w_bias, b):
    x = np.asarray(x)
    w_packed = np.asarray(w_packed)
    w_scale = np.asarray(w_scale)
    w_bias = np.asarray(w_bias)
    b = np.asarray(b)
    y, _ = _run(x, w_packed, w_scale, w_bias, b, trace=False)
    return y
